# revision 7
# baseline (speedup 1.0000x reference)
"""Bass/Trainium2 kernel for nn_D_constraint1: 0.001*sqrt(sum_i (||d_i||^2 - 1)^2).

Sharding: d [16384, 2048] is split row-wise across 8 NeuronCores (2048 rows
each). Each core computes sum over its rows of (||row||^2 - 1)^2, reduced to a
[128, 1] per-partition partial. The host gathers the 8x128 partials, sums,
takes sqrt and scales — the scalar "all-reduce" of the sharding hint.
"""

import numpy as np

import concourse.bass as bass
import concourse.tile as tile
from concourse import bacc, mybir
from concourse.bass_utils import run_bass_kernel_spmd

N, K = 16384, 2048
NCORES = 8
R = N // NCORES  # rows per core
P = 128          # SBUF partitions
T = R // P       # row-tiles per core

_nc_cache = None


def _build_nc():
    f32 = mybir.dt.float32
    nc = bacc.Bacc("TRN2", target_bir_lowering=False, debug=False)
    d = nc.dram_tensor("d", [R, K], f32, kind="ExternalInput").ap()
    out = nc.dram_tensor("out", [P, 1], f32, kind="ExternalOutput").ap()
    Square = mybir.ActivationFunctionType.Square

    with tile.TileContext(nc) as tc:
        with (
            tc.tile_pool(name="inp", bufs=4) as inp,
            tc.tile_pool(name="sq", bufs=2) as sqp,
            tc.tile_pool(name="stat", bufs=1) as stat,
        ):
            s = stat.tile([P, T], f32)  # per-row ||row||^2, one column per tile
            neg1 = stat.tile([P, 1], f32)
            nc.gpsimd.memset(neg1[:], -1.0)
            for i in range(T):
                t = inp.tile([P, K], f32)
                nc.sync.dma_start(t[:], d[i * P : (i + 1) * P, :])
                junk = sqp.tile([P, K], f32)
                nc.scalar.activation(junk[:], t[:], Square, accum_out=s[:, i : i + 1])
            junk2 = stat.tile([P, T], f32)
            partial = stat.tile([P, 1], f32)
            # partial[p] = sum_i (s[p,i] - 1)^2
            nc.scalar.activation(
                junk2[:], s[:], Square, bias=neg1[:], scale=1.0, accum_out=partial[:]
            )
            nc.sync.dma_start(out, partial[:])
    nc.compile()
    return nc


def _build_nc_raw():
    """Raw Bacc version: manual semaphores, no Tile drain/barrier tail.

    SP issues all 16 input-tile DMAs up front (each lands in its own SBUF
    region), ACT squares+row-sums each tile as its DMA completes, then one
    more ACT computes (1-s)^2 summed -> [128,1] partial, which SP DMAs out.
    """
    f32 = mybir.dt.float32
    nc = bacc.Bacc("TRN2", target_bir_lowering=False, debug=False)
    d = nc.dram_tensor("d", [R, K], f32, kind="ExternalInput").ap()
    out = nc.dram_tensor("out", [P, 1], f32, kind="ExternalOutput").ap()
    Square = mybir.ActivationFunctionType.Square

    with (
        nc.semaphore("dma_sem") as dma_sem,
        nc.semaphore("act_sem") as act_sem,
        nc.semaphore("outd_sem") as outd_sem,
        nc.sbuf_tensor("t", [P, T * K], f32) as t,
        nc.sbuf_tensor("junk", [P, K], f32) as junk,
        nc.sbuf_tensor("s", [P, T], f32) as s,
        nc.sbuf_tensor("partial", [P, 1], f32) as partial,
    ):
        with nc.Block() as block:

            @block.sync
            def _(sync):
                for i in range(T):
                    sync.dma_start(
                        out=t.ap()[:, i * K : (i + 1) * K], in_=d[i * P : (i + 1) * P, :]
                    ).then_inc(dma_sem, 16)
                sync.wait_ge(act_sem, T + 1)
                sync.dma_start(out=out, in_=partial.ap()).then_inc(outd_sem, 16)
                sync.wait_ge(outd_sem, 16)

            @block.scalar
            def _(scalar):
                for i in range(T):
                    scalar.wait_ge(dma_sem, (i + 1) * 16)
                    scalar.activation(
                        junk.ap(),
                        t.ap()[:, i * K : (i + 1) * K],
                        Square,
                        accum_out=s.ap()[:, i : i + 1],
                    ).then_inc(act_sem)
                # partial[p] = sum_i (1 - s[p,i])^2 == sum_i (s[p,i] - 1)^2
                scalar.activation(
                    junk.ap()[:, 0:T],
                    s.ap(),
                    Square,
                    bias=1.0,
                    scale=-1.0,
                    accum_out=partial.ap(),
                ).then_inc(act_sem)

    nc.compile()
    return nc


def _get_nc():
    global _nc_cache
    if _nc_cache is None:
        _nc_cache = _build_nc_raw()
    return _nc_cache


def run_shards(d, **spmd_kwargs):
    """Run the SPMD kernel; returns the BassKernelResults (for profiling)."""
    d = np.ascontiguousarray(np.asarray(d, dtype=np.float32))
    assert d.shape == (N, K), d.shape
    shards = d.reshape(NCORES, R, K)
    in_maps = [{"d": shards[c]} for c in range(NCORES)]
    return run_bass_kernel_spmd(_get_nc(), in_maps, list(range(NCORES)), **spmd_kwargs)

def _combine(results):
    total = 0.0
    for r in results:
        total += np.sum(r["out"].astype(np.float64))
    return np.float32(0.001 * np.sqrt(total))


def kernel(d):
    return _combine(run_shards(d).results)


# revision 23
# speedup vs baseline: 1.0867x; 1.0867x over previous
"""Bass/Trainium2 kernel for nn_D_constraint1: 0.001*sqrt(sum_i (||d_i||^2 - 1)^2).

Sharding: d [16384, 2048] is split row-wise across 8 NeuronCores (2048 rows
each; the row dimension is fully parallel per the sharding hint). Each core
streams its 16 MiB shard HBM->SBUF in 16 [128,2048] tiles and computes per-row
sums of squares on the fly: odd tiles on the scalar engine (Square activation
with free-axis accumulator), even tiles on the vector engine (square then
pool-average). Two more scalar-engine activations fold the per-row sums into
sum (1-s)^2 per partition. The host gathers the per-core partials, sums,
takes sqrt and scales - the scalar "all-reduce" of the sharding hint.
"""

from contextlib import ExitStack

import numpy as np

import concourse.bass as bass
from concourse import bacc, mybir
from concourse.bass_utils import run_bass_kernel_spmd

N, K = 16384, 2048
NCORES = 8
R = N // NCORES  # rows per core
P = 128          # SBUF partitions
T = R // P       # row-tiles per core

_nc_cache = None


def _build_nc_v3(wait_out="wait", reduce_mode="none"):
    f32 = mybir.dt.float32
    nc = bacc.Bacc("TRN2", target_bir_lowering=False, debug=False)
    d = nc.dram_tensor("d", [R, K], f32, kind="ExternalInput").ap()
    out_shape = [1, 2] if reduce_mode == "gpsimd" else [P, 2]
    out = nc.dram_tensor("out", out_shape, f32, kind="ExternalOutput").ap()
    Square = mybir.ActivationFunctionType.Square

    act_tiles = list(range(1, T, 2))  # ACT gets the last tile (shorter tail)
    dve_tiles = list(range(0, T, 2))
    NA, NV = len(act_tiles), len(dve_tiles)

    ctx = ExitStack()
    dsem = [ctx.enter_context(nc.semaphore(f"dma_{i}")) for i in range(T)]
    with (
        ctx,
        nc.semaphore("act_sem") as act_sem,
        nc.semaphore("dve_sem") as dve_sem,
        nc.semaphore("pool_sem") as pool_sem,
        nc.semaphore("outd_sem") as outd_sem,
        nc.semaphore("dummy_sem") as dummy_sem,
        nc.sbuf_tensor("t", [P, T * K], f32) as t,
        nc.sbuf_tensor("junk_a", [P, K], f32) as junk_a,
        nc.sbuf_tensor("sq_v", [P, K], f32) as sq_v,
        nc.sbuf_tensor("s_a", [P, NA], f32) as s_a,
        nc.sbuf_tensor("s_v", [P, NV], f32) as s_v,
        nc.sbuf_tensor("partial", [P, 2], f32) as partial,
        nc.sbuf_tensor("red", [P, 2], f32) as red,
        nc.sbuf_tensor("scratch", [1, 1], f32) as scratch,
    ):
        with nc.Block() as block:

            @block.sync
            def _(sync):
                for i in range(T):
                    sync.dma_start(
                        out=t.ap()[:, i * K : (i + 1) * K],
                        in_=d[i * P : (i + 1) * P, :],
                    ).then_inc(dsem[i], 16)
                if reduce_mode == "gpsimd":
                    sync.wait_ge(pool_sem, 1)
                    out_src = red.ap()[0:1, :]
                else:
                    sync.wait_ge(act_sem, NA + 2)
                    out_src = partial.ap()
                sync.dma_start(out=out, in_=out_src).then_inc(outd_sem, 16)
                if wait_out == "flush":
                    sync.dma_start(out=scratch.ap(), in_=d[0:1, 0:1]).then_inc(
                        dummy_sem, 16
                    )
                    sync.wait_ge(outd_sem, 16)
                elif wait_out == "wait":
                    sync.wait_ge(outd_sem, 16)
                # "none": SP stream just ends; program epilogue drains DMA

            @block.scalar
            def _(scalar):
                for j, i in enumerate(act_tiles):
                    scalar.wait_ge(dsem[i], 16)
                    if j > 0:
                        scalar.wait_ge(act_sem, j)
                    scalar.activation(
                        junk_a.ap(),
                        t.ap()[:, i * K : (i + 1) * K],
                        Square,
                        accum_out=s_a.ap()[:, j : j + 1],
                    ).then_inc(act_sem)
                scalar.wait_ge(act_sem, NA)
                scalar.wait_ge(dve_sem, 2 * NV)
                # sum_j (1 - s_a[p,j])^2 over ACT's tiles
                scalar.activation(
                    junk_a.ap()[:, 0:NA],
                    s_a.ap(),
                    Square,
                    bias=1.0,
                    scale=-1.0,
                    accum_out=partial.ap()[:, 0:1],
                ).then_inc(act_sem)
                scalar.wait_ge(act_sem, NA + 1)
                # s_v holds means: sum_j (1 - K*mean)^2 over DVE's tiles
                scalar.activation(
                    junk_a.ap()[:, NA : NA + NV],
                    s_v.ap(),
                    Square,
                    bias=1.0,
                    scale=-float(K),
                    accum_out=partial.ap()[:, 1:2],
                ).then_inc(act_sem)

            @block.vector
            def _(vector):
                for j, i in enumerate(dve_tiles):
                    vector.wait_ge(dsem[i], 16)
                    if j > 0:
                        vector.wait_ge(dve_sem, 2 * j)
                    vector.tensor_mul(
                        sq_v.ap(),
                        t.ap()[:, i * K : (i + 1) * K],
                        t.ap()[:, i * K : (i + 1) * K],
                    ).then_inc(dve_sem)
                    vector.wait_ge(dve_sem, 2 * j + 1)
                    vector.pool_avg(s_v.ap()[:, j : j + 1], sq_v.ap()).then_inc(
                        dve_sem
                    )

            if reduce_mode == "gpsimd":
                import concourse.bass_isa as bass_isa

                @block.gpsimd
                def _(gpsimd):
                    gpsimd.wait_ge(act_sem, NA + 2)
                    gpsimd.partition_all_reduce(
                        red.ap(),
                        partial.ap(),
                        channels=P,
                        reduce_op=bass_isa.ReduceOp.add,
                    ).then_inc(pool_sem)

    nc.compile()
    return nc


def _build_nc_v4(dve_tiles=(0, 3, 6, 9, 12), wait_out="none", reduce_mode="gpsimd"):
    """Weighted ACT/DVE split (ACT ~2.3us/tile vs DVE ~4.6us/tile), single
    final activation (DVE rescales its pool-averages to sums in place),
    gpsimd cross-partition reduce -> 4-byte output DMA, no completion wait
    (the program epilogue's drain guarantees the write lands before exec
    completes)."""
    import concourse.bass_isa as bass_isa

    f32 = mybir.dt.float32
    nc = bacc.Bacc("TRN2", target_bir_lowering=False, debug=False)
    d = nc.dram_tensor("d", [R, K], f32, kind="ExternalInput").ap()
    out_shape = [1, 1] if reduce_mode == "gpsimd" else [P, 1]
    out = nc.dram_tensor("out", out_shape, f32, kind="ExternalOutput").ap()
    Square = mybir.ActivationFunctionType.Square

    dve_tiles = list(dve_tiles)
    act_tiles = [i for i in range(T) if i not in dve_tiles]
    NA, NV = len(act_tiles), len(dve_tiles)

    ctx = ExitStack()
    dsem = [ctx.enter_context(nc.semaphore(f"dma_{i}")) for i in range(T)]
    with (
        ctx,
        nc.semaphore("act_sem") as act_sem,
        nc.semaphore("dve_sem") as dve_sem,
        nc.semaphore("pool_sem") as pool_sem,
        nc.semaphore("outd_sem") as outd_sem,
        nc.sbuf_tensor("t", [P, T * K], f32) as t,
        nc.sbuf_tensor("junk_a", [P, K], f32) as junk_a,
        nc.sbuf_tensor("sq_v", [P, K], f32) as sq_v,
        nc.sbuf_tensor("s", [P, T], f32) as s,
        nc.sbuf_tensor("partial", [P, 1], f32) as partial,
        nc.sbuf_tensor("red", [P, 1], f32) as red,
    ):
        with nc.Block() as block:

            @block.sync
            def _(sync):
                for i in range(T):
                    sync.dma_start(
                        out=t.ap()[:, i * K : (i + 1) * K],
                        in_=d[i * P : (i + 1) * P, :],
                    ).then_inc(dsem[i], 16)
                if reduce_mode == "gpsimd":
                    sync.wait_ge(pool_sem, 1)
                    out_src = red.ap()[0:1, :]
                else:
                    sync.wait_ge(act_sem, NA + 1)
                    out_src = partial.ap()
                sync.dma_start(out=out, in_=out_src).then_inc(outd_sem, 16)
                if wait_out == "wait":
                    sync.wait_ge(outd_sem, 16)

            @block.scalar
            def _(scalar):
                for j, i in enumerate(act_tiles):
                    scalar.wait_ge(dsem[i], 16)
                    if j > 0:
                        scalar.wait_ge(act_sem, j)
                    scalar.activation(
                        junk_a.ap(),
                        t.ap()[:, i * K : (i + 1) * K],
                        Square,
                        accum_out=s.ap()[:, i : i + 1],
                    ).then_inc(act_sem)
                scalar.wait_ge(act_sem, NA)
                scalar.wait_ge(dve_sem, 3 * NV)
                scalar.activation(
                    junk_a.ap()[:, 0:T],
                    s.ap(),
                    Square,
                    bias=1.0,
                    scale=-1.0,
                    accum_out=partial.ap(),
                ).then_inc(act_sem)

            @block.vector
            def _(vector):
                for j, i in enumerate(dve_tiles):
                    vector.wait_ge(dsem[i], 16)
                    if j > 0:
                        vector.wait_ge(dve_sem, 3 * j)
                    vector.tensor_mul(
                        sq_v.ap(),
                        t.ap()[:, i * K : (i + 1) * K],
                        t.ap()[:, i * K : (i + 1) * K],
                    ).then_inc(dve_sem)
                    vector.wait_ge(dve_sem, 3 * j + 1)
                    vector.pool_avg(s.ap()[:, i : i + 1], sq_v.ap()).then_inc(dve_sem)
                    vector.wait_ge(dve_sem, 3 * j + 2)
                    vector.tensor_scalar_mul(
                        s.ap()[:, i : i + 1], s.ap()[:, i : i + 1], float(K)
                    ).then_inc(dve_sem)

            if reduce_mode == "gpsimd":

                @block.gpsimd
                def _(gpsimd):
                    gpsimd.wait_ge(act_sem, NA + 1)
                    gpsimd.partition_all_reduce(
                        red.ap(),
                        partial.ap(),
                        channels=P,
                        reduce_op=bass_isa.ReduceOp.add,
                    ).then_inc(pool_sem)

    nc.compile()
    return nc


def _get_nc():
    global _nc_cache
    if _nc_cache is None:
        _nc_cache = _build_nc_v4()
    return _nc_cache


def run_shards(d, **spmd_kwargs):
    """Run the SPMD kernel; returns the BassKernelResults (for profiling)."""
    d = np.ascontiguousarray(np.asarray(d, dtype=np.float32))
    assert d.shape == (N, K), d.shape
    shards = d.reshape(NCORES, R, K)
    in_maps = [{"d": shards[c]} for c in range(NCORES)]
    return run_bass_kernel_spmd(_get_nc(), in_maps, list(range(NCORES)), **spmd_kwargs)


def _combine(results):
    total = 0.0
    for r in results:
        total += float(np.sum(r["out"].astype(np.float64)))
    return np.float32(0.001 * np.sqrt(total))


def kernel(d):
    return _combine(run_shards(d).results)
